# revision 20
# baseline (speedup 1.0000x reference)
"""Teacher-forced Elman RNN decoder on 8 Trainium2 NeuronCores.

Math per time step t (T=512 steps, serial recurrence):
    h = tanh(x_t @ W_ih.T + b_ih + h @ W_hh.T + b_hh)     # [B, H], H=1024
    y_t = h @ W_out.T + b_out                              # [B, 1]

Sharding: 2D (batch x time). The 8 cores form 2 batch-groups (128 rows
each) x 4 time-quarters. Teacher forcing means every step's input x_t is
known up front (targets[t-1]), so later time-quarters start BURN steps
early from h=0: the tanh recurrence forgets its initial state (HW/np
measured: 5e-6 err after 16 steps, 3e-8 after 32) and the first BURN
outputs are discarded. Each core runs t_core = T/4 + BURN steps with
N=128 matmul free dim. Weights replicated; no collectives.

Per-core kernel design (all operands SBUF-resident, no per-step HBM reads):
  * State TRANSPOSED: hT[p, k*128+b] = h[b, k*128+p]  (k = 0..7 H-chunks).
  * Main matmuls: psum[o] += Wt(k,o).T @ hT_k, Wt(k,o)[p,m] =
    W_hh[o*128+m, k*128+p]. Output is h_nextT directly.
  * PSUM: per-step ring of 3 slots x 2 banks (slot j%3 holds all 8 output
    chunks of step j: chunk o -> cols o*128 of the slot's [P,1024] tile).
    ACT of step j reads slot j%3 while PE accumulates slot (j+1)%3 --
    different banks, no PSUM port contention (the old 2-step-phase layout
    had ACT and PE on the same bank and measured ~900ns/step of stall).
  * Seeds (x W_ih + b_ih + b_hh) land 2 steps ahead: 2 matmuls of N=512,
    K=8 with a block-diagonal rhs [8,512] prepared host-side per step
    ([x_t;1] per 4-chunk block), lhsT rows = [W_ih;c] per chunk. One
    32KB x-DMA per 4 steps into a 2-slot rhs ring (fewer wait-carrying
    PE instructions).
  * y-projection in-loop (no epilogue): DVE computes z[p,b] =
    sum_o W_out[o*128+p]*hT[p,o*128+b] as one mul + contiguous add-tree
    (1024->512->256->128 cols, fp16 out), then a single ones[128,1] fp16
    matmul reduces over partitions into a dedicated 2-bank y PSUM ring
    (4 matmuls batched per 4 steps, one DVE wait); DVE copies [1,512]
    out every 4 steps, DMA to y_d. This replaced a 40-matmul fp32
    epilogue over a 10.5MB zbuf DRAM round trip.

Hardware loop: For_i over niter iterations of U=24 unrolled steps
(U % 12 == 0 keeps the mod-3 psum ring and mod-4 x ring aligned).

Performance notes (HW-measured):
  * Cost model (mm_probe): every [128x128]xN matmul costs ~22 ns fixed +
    N/2.4GHz streaming, weight dtype irrelevant (LDWEIGHTS hidden).
    Per step: 64 main MMs (N=128) 4821 + 2 seed MMs (N=512) 470 +
    y MM 75 = ~5366 ns theoretical.
  * Differential builds (HW): pure 64-MM stream runs 4411 ns/step;
    +ACT w/o recurrence chain 4611; +chain 6284 with 4 pair-ACTs,
    5430 with 2 half-ACTs -- each wait-carrying PE matmul (cross-step
    ACT->PE edge) costs ~430 ns (sem check + 173 ns SBUF-pipeline
    refill + decode break), so fewer, coarser edges win. One whole-step
    ACT would serialize ~1.2 us into the chain; two is the sweet spot.
  * History: fp16 B=32 baseline 1.357 ms -> y add-tree 2396 ns/step ->
    2-way time split B=64 1.015 ms -> 4-way split B=128 2-step phases
    6166 ns/step x 160 = 986 us (this file's predecessor).
  * Tried and rejected (HW-measured): fp8e3 weights (no gain, not
    weight-load bound); explicit ldweights (2x worse); U=128 (no change);
    8-way split w/ 1-step phases (13.7 us/step -- seed MMs on critical
    path); cross-core H-split (D2D 0.5-2us/hop >> step budget).
"""

import numpy as np

import concourse.bass as bass
import concourse.bacc as bacc
import concourse.mybir as mybir
import concourse.tile as tile
from concourse.bass_utils import run_bass_kernel_spmd

P = 128          # partitions
B = 128          # local batch (2 batch-groups x 4 time-quarters)
H = 1024
NCH = 8          # H / P chunks
T_FULL = 512
BURN = 16        # time-split burn-in steps (err ~5e-6 at 16, HW-measured)
U = 24           # steps per hardware-loop iteration (must be % 12 == 0)
N_CORES = 8

DT_MAIN = mybir.dt.float16
NP_MAIN = np.float16

_CACHE = {}


def _build(t_total: int, debug: bool, repeat: int = 1):
    """Build the Bass program for one core. Returns nc."""
    key = (t_total, debug, DT_MAIN, repeat)
    if key in _CACHE:
        return _CACHE[key]

    assert t_total % U == 0
    niter = t_total // U
    f32 = mybir.dt.float32

    nc = bacc.Bacc("TRN2", target_bir_lowering=False, debug=debug)

    # ---- DRAM I/O ----
    w_d = nc.dram_tensor("w", [P, NCH * NCH * P], DT_MAIN, kind="ExternalInput")
    # seed lhsT: rows 2i/2i+1 = [W_ih; c] of chunk 4s+i, cols s*128+m
    seedw_d = nc.dram_tensor("seedw", [8, 2 * P], DT_MAIN, kind="ExternalInput")
    # block-diag seed rhs per step: [8, 512], rows 2i/2i+1 = [x_t; 1] at
    # col block i (chunk 4s+i), zeros elsewhere; padded 8 extra steps
    x_d = nc.dram_tensor("x", [8, (t_total + 8) * 512], DT_MAIN,
                         kind="ExternalInput")
    h0_d = nc.dram_tensor("h0", [H, B], DT_MAIN, kind="ExternalInput")
    # wout pre-broadcast to the hT layout: wout[p, o*128+b] = W_out[o*128+p]
    wout_d = nc.dram_tensor("wout", [P, NCH * B], f32, kind="ExternalInput")
    # 512-col guard at the front: the y pipeline runs one step behind the
    # recurrence, so the first iteration's copy lands at offset -512.
    y_d = nc.dram_tensor("y", [1, t_total * B + 512], f32,
                         kind="ExternalOutput")

    with tile.TileContext(nc) as tc:
        with (
            tc.tile_pool(name="pool", bufs=1) as pool,
            tc.tile_pool(name="psum", bufs=1, space=bass.MemorySpace.PSUM) as psum,
        ):
            w_sb = pool.tile([P, NCH * NCH * P], DT_MAIN, tag="w")
            seedw_sb = pool.tile([8, 2 * P], DT_MAIN, tag="seedw")
            wout_sb = pool.tile([P, NCH * B], f32, tag="wout")
            hT = [
                pool.tile([P, NCH * B], DT_MAIN, tag="hT0", name="hT0"),
                pool.tile([P, NCH * B], DT_MAIN, tag="hT1", name="hT1"),
            ]
            prod = pool.tile([P, NCH * B], f32, tag="prod")
            z_sb = pool.tile([P, 8 * B], DT_MAIN, tag="z")   # 8 step slots
            y_sb = pool.tile([1, 2 * 512], f32, tag="ysb")   # 2 4-step slots
            # x ring: 2 slots of 4 steps each (one 32KB DMA per 4 steps)
            x_ring = pool.tile([8, 2 * 2048], DT_MAIN, tag="xring")
            ones_sb = pool.tile([P, 1], DT_MAIN, tag="ones")

            # 3-slot main psum ring (2 banks each) + 2-bank y ring = 8 banks
            mb = [
                psum.tile([P, NCH * B], f32, tag=f"mb{i}", name=f"mb{i}")
                for i in range(3)
            ]
            y_ps = psum.tile([P, 1024], f32, tag="yps", name="yps")

            def seed_mms(slot, xcol0):
                """Seed psum slot (one future step) with W_ih x + bias.
                One start=True matmul per bank (N=512, K=8)."""
                for s in range(2):
                    nc.tensor.matmul(
                        mb[slot][:, s * 512:(s + 1) * 512],
                        seedw_sb[:, s * P:(s + 1) * P],
                        x_ring[:, xcol0:xcol0 + 512],
                        start=True,
                        stop=False,
                        skip_group_check=True,
                    )

            # ---- prologue ----
            nc.sync.dma_start(out=w_sb[:], in_=w_d[:])
            nc.sync.dma_start(out=seedw_sb[:], in_=seedw_d[:])
            nc.sync.dma_start(out=wout_sb[:], in_=wout_d[:])
            nc.sync.dma_start(
                out=hT[0][:].rearrange("p (k b) -> p k b", k=NCH),
                in_=h0_d[:].rearrange("(k p) b -> p k b", p=P),
            )
            nc.gpsimd.memset(ones_sb[:], 1.0)
            nc.gpsimd.memset(z_sb[:], 0.0)
            for s in range(2):
                nc.sync.dma_start(
                    out=x_ring[:, s * 2048:(s + 1) * 2048],
                    in_=x_d[:, s * 2048:(s + 1) * 2048],
                )
            seed_mms(0, 0)      # step 0
            seed_mms(1, 512)    # step 1

            # ---- main loop ----
            from contextlib import nullcontext

            rep_cm = (tc.For_i(0, repeat, 1, name="rep") if repeat > 1
                      else nullcontext(0))
            with rep_cm, tc.For_i(
                    0, niter, 1, hint_engines=(mybir.EngineType.PE,)) as g:
                for j in range(U):
                    slot = j % 3
                    h_in = hT[j % 2]
                    h_out = hT[(j + 1) % 2]

                    # x prefetch: one 32KB DMA per 4 steps (group j//4 + 2),
                    # issued at j%4==2, right after the slot's last seed read
                    if j % 4 == 2:
                        xs0 = ((j // 4) % 2) * 2048
                        nc.sync.dma_start(
                            out=x_ring[:, xs0:xs0 + 2048],
                            in_=x_d[:, bass.ds(
                                g * (U * 512) + (4 * (j // 4) + 8) * 512, 2048)],
                        )

                    # h_preT[o] += sum_k Wt(k,o).T @ hT_k, split into two
                    # fixed phases: Phase-E contracts chunks 0-3 (tanh'd
                    # EARLY in step j-1, mid-Phase-L), Phase-L chunks 4-7
                    # (tanh'd at step j-1's end).  tanh fires as TWO
                    # half-ACTs (512 cols = one psum bank each) inside
                    # Phase-L as soon as a half's output pairs finish.  Each
                    # wait-carrying PE matmul costs ~430ns (HW-measured), so
                    # two coarse cross-step edges instead of four, and the
                    # phase split gives both >= 1us of slack (the old
                    # interleaved k_order consumed late-half chunks at MM#9,
                    # ~620ns in, vs producer ready ~640ns -- marginal every
                    # step).  One whole-step ACT would serialize ~1.2us, so
                    # two is the sweet spot.
                    for a in range(4):
                        for k in range(4):
                            for half in range(2):
                                o = 2 * a + half
                                nc.tensor.matmul(
                                    mb[slot][:, o * P:(o + 1) * P],
                                    w_sb[:, (k * NCH + o) * P:(k * NCH + o + 1) * P],
                                    h_in[:, k * P:(k + 1) * P],
                                    start=False,
                                    stop=False,
                                    skip_group_check=True,
                                )

                    # seeds for step j+2, emitted between the phases: their
                    # WAR wait (slot (j+2)%3 was read by ACT(j-1)) rides the
                    # same ACT semaphore count Phase-L needs, and by
                    # mid-step it is long satisfied
                    seed_mms((j + 2) % 3,
                             (((j + 2) // 4) % 2) * 2048 + ((j + 2) % 4) * 512)

                    for a in range(4):
                        for k in range(4, 8):
                            for half in range(2):
                                o = 2 * a + half
                                nc.tensor.matmul(
                                    mb[slot][:, o * P:(o + 1) * P],
                                    w_sb[:, (k * NCH + o) * P:(k * NCH + o + 1) * P],
                                    h_in[:, k * P:(k + 1) * P],
                                    start=False,
                                    stop=(k == 7),
                                    skip_group_check=True,
                                )
                        if a == 1:
                            nc.scalar.activation(
                                h_out[:, 0:512],
                                mb[slot][:, 0:512],
                                mybir.ActivationFunctionType.Tanh,
                            )
                        elif a == 3:
                            nc.scalar.activation(
                                h_out[:, 512:1024],
                                mb[slot][:, 512:1024],
                                mybir.ActivationFunctionType.Tanh,
                            )

                    # z[p, b] = sum_o wout[p, o*128+b] * h_out[p, o*128+b]
                    nc.vector.tensor_mul(prod[:], wout_sb[:], h_out[:])
                    nc.vector.tensor_add(
                        prod[:, 0:512], prod[:, 0:512], prod[:, 512:1024])
                    nc.vector.tensor_add(
                        prod[:, 0:256], prod[:, 0:256], prod[:, 256:512])
                    zc = (j % 8) * B
                    nc.vector.tensor_add(
                        z_sb[:, zc:zc + B], prod[:, 0:B], prod[:, B:2 * B])
                    if j % 4 == 0:
                        # partition-reduce z of steps j-4..j-1 on PE (a full
                        # step behind the DVE tree; only the first matmul of
                        # the burst carries the DVE wait). Requires U%8==0.
                        b0 = ((j // 4 + 1) % 2) * 4
                        for r in range(4):
                            nc.tensor.matmul(
                                y_ps[0:1, (b0 + r) * B:(b0 + r + 1) * B],
                                ones_sb[:, 0:1],
                                z_sb[:, (b0 + r) * B:(b0 + r + 1) * B],
                                start=True,
                                stop=True,
                            )
                        bh = b0 // 4
                        # copy on DVE, not ACT: keeps the ACT queue holding
                        # only the two critical-path tanh instructions
                        nc.vector.tensor_copy(
                            y_sb[0:1, bh * 512:bh * 512 + 512],
                            y_ps[0:1, bh * 512:bh * 512 + 512],
                        )
                        nc.sync.dma_start(
                            out=y_d[0:1, bass.ds(
                                g * (U * B) + (j - 4) * B + 512, 512)],
                            in_=y_sb[0:1, bh * 512:bh * 512 + 512],
                        )

            # ---- tiny epilogue: y for the final 4 steps ----
            for r in range(4):
                nc.tensor.matmul(
                    y_ps[0:1, (4 + r) * B:(4 + r + 1) * B],
                    ones_sb[:, 0:1],
                    z_sb[:, (4 + r) * B:(4 + r + 1) * B],
                    start=True,
                    stop=True,
                )
            nc.vector.tensor_copy(
                y_sb[0:1, 512:1024],
                y_ps[0:1, 512:1024],
            )
            nc.sync.dma_start(
                out=y_d[0:1, (t_total - 4) * B + 512:t_total * B + 512],
                in_=y_sb[0:1, 512:1024],
            )

    nc.compile()
    _CACHE[key] = nc
    return nc


def _prep_inputs(initial_input, hidden, targets, W_ih, W_hh, b_ih, b_hh,
                 W_out, t_full):
    """Host-side prep: returns the 8 per-core input maps.

    Core ci = (ts, bg): time-quarter ts = ci // 2, batch-group bg = ci % 2.
    ts=0 runs steps [0, t_core); ts>0 runs steps [ts*t_full/4 - BURN, ...),
    starting from h=0 -- the teacher-forced recurrence forgets its initial
    state (err ~5e-6 after 16 steps), so the first BURN outputs of ts>0
    are discarded.
    """
    f32 = np.float32
    t_core = t_full // 4 + BURN
    # x sequence: teacher-forced input at step t is targets[t-1], x_0 = initial
    x_seq = np.concatenate(
        [np.asarray(initial_input, f32).reshape(1, -1),
         np.asarray(targets, f32)[: t_full - 1, :, 0]],
        axis=0,
    )  # [T, 256]
    c_bias = (np.asarray(b_ih, f32) + np.asarray(b_hh, f32))
    # w_sb[p, (k*8+o)*128+m] = W_hh.T[k*128+p, o*128+m]
    wt = (
        np.asarray(W_hh, f32).T.reshape(NCH, P, NCH, P)
        .transpose(1, 0, 2, 3)
        .reshape(P, NCH * NCH * P)
        .astype(NP_MAIN)
    )
    # seedw[2i+t, s*128+m] = [W_ih; c][t] at H-index (4s+i)*128+m
    wih = np.asarray(W_ih, f32)[:, 0].reshape(NCH, P)
    cb = c_bias.reshape(NCH, P)
    seedw = np.zeros((8, 2 * P), f32)
    for s in range(2):
        for i in range(4):
            seedw[2 * i, s * P:(s + 1) * P] = wih[4 * s + i]
            seedw[2 * i + 1, s * P:(s + 1) * P] = cb[4 * s + i]
    seedw = seedw.astype(NP_MAIN)
    wout8 = np.asarray(W_out, f32)[0].reshape(NCH, P).T                  # [128, 8]
    wout = np.ascontiguousarray(
        np.broadcast_to(wout8[:, :, None], (P, NCH, B)).reshape(P, NCH * B)
    )

    in_maps = []
    for ci in range(N_CORES):
        ts, bg = ci // 2, ci % 2
        sl = slice(bg * B, (bg + 1) * B)
        start = 0 if ts == 0 else (ts * (t_full // 4) - BURN)
        xs = x_seq[start:start + t_core, sl].astype(NP_MAIN)  # [t_core, 128]
        xblk = np.zeros((8, (t_core + 8) * 512), NP_MAIN)
        for i in range(4):
            v = xblk[2 * i].reshape(t_core + 8, 4, P)
            v[:t_core, i, :] = xs
            xblk[2 * i + 1].reshape(t_core + 8, 4, P)[:, i, :] = 1.0
        if ts == 0:
            h0 = np.ascontiguousarray(
                np.asarray(hidden, f32)[sl].T).astype(NP_MAIN)
        else:
            h0 = np.zeros((H, B), NP_MAIN)
        in_maps.append({
            "w": wt, "seedw": seedw, "x": xblk, "h0": h0, "wout": wout,
        })
    return in_maps


def kernel(initial_input, hidden, targets, W_ih, W_hh, b_ih, b_hh, W_out,
           b_out, teacher_force_probability=None, _trace=False):
    t_full = int(np.asarray(targets).shape[0])
    t_core = t_full // 4 + BURN
    nc = _build(t_core, debug=False)
    in_maps = _prep_inputs(initial_input, hidden, targets, W_ih, W_hh, b_ih,
                           b_hh, W_out, t_full)
    res = run_bass_kernel_spmd(nc, in_maps, core_ids=list(range(N_CORES)),
                               trace=_trace)
    y = np.zeros((t_full, 256), np.float32)
    q4 = t_full // 4
    for ci, r in enumerate(res.results):
        ts, bg = ci // 2, ci % 2
        yc = r["y"].reshape(-1)[512:].reshape(t_core, B).astype(np.float32)
        if ts == 0:
            y[:q4, bg * B:(bg + 1) * B] = yc[:q4]
        else:
            y[ts * q4:(ts + 1) * q4, bg * B:(bg + 1) * B] = yc[BURN:]
    y = y + np.float32(np.asarray(b_out).reshape(-1)[0])
    out = y[:, :, None]
    if _trace:
        return out, res
    return out


# revision 21
# speedup vs baseline: 1.0570x; 1.0570x over previous
"""Teacher-forced Elman RNN decoder on 8 Trainium2 NeuronCores.

Math per time step t (T=512 steps, serial recurrence):
    h = tanh(x_t @ W_ih.T + b_ih + h @ W_hh.T + b_hh)     # [B, H], H=1024
    y_t = h @ W_out.T + b_out                              # [B, 1]

Sharding: 2D (batch x time). The 8 cores form 2 batch-groups (128 rows
each) x 4 time-quarters. Teacher forcing means every step's input x_t is
known up front (targets[t-1]), so later time-quarters start BURN steps
early from h=0: the tanh recurrence forgets its initial state (HW/np
measured: 5e-6 err after 16 steps, 3e-8 after 32) and the first BURN
outputs are discarded. Each core runs t_core = T/4 + BURN steps with
N=128 matmul free dim. Weights replicated; no collectives.

Per-core kernel design (all operands SBUF-resident, no per-step HBM reads):
  * State TRANSPOSED: hT[p, k*128+b] = h[b, k*128+p]  (k = 0..7 H-chunks).
  * Main matmuls: psum[o] += Wt(k,o).T @ hT_k, Wt(k,o)[p,m] =
    W_hh[o*128+m, k*128+p]. Output is h_nextT directly.
  * PSUM: per-step ring of 3 slots x 2 banks (slot j%3 holds all 8 output
    chunks of step j: chunk o -> cols o*128 of the slot's [P,1024] tile).
    ACT of step j reads slot j%3 while PE accumulates slot (j+1)%3 --
    different banks, no PSUM port contention (the old 2-step-phase layout
    had ACT and PE on the same bank and measured ~900ns/step of stall).
  * Seeds (x W_ih + b_ih + b_hh) land 2 steps ahead: 2 matmuls of N=512,
    K=8 with a block-diagonal rhs [8,512] prepared host-side per step
    ([x_t;1] per 4-chunk block), lhsT rows = [W_ih;c] per chunk. One
    32KB x-DMA per 4 steps into a 2-slot rhs ring (fewer wait-carrying
    PE instructions).
  * y-projection in-loop (no epilogue): DVE computes z[p,b] =
    sum_o W_out[o*128+p]*hT[p,o*128+b] as one mul + contiguous add-tree
    (1024->512->256->128 cols, fp16 out), then a single ones[128,1] fp16
    matmul reduces over partitions into a dedicated 2-bank y PSUM ring
    (4 matmuls batched per 4 steps, one DVE wait); DVE copies [1,512]
    out every 4 steps, DMA to y_d. This replaced a 40-matmul fp32
    epilogue over a 10.5MB zbuf DRAM round trip.

Hardware loop: For_i over niter iterations of U=24 unrolled steps
(U % 12 == 0 keeps the mod-3 psum ring and mod-4 x ring aligned).

Performance notes (HW-measured):
  * Cost model (mm_probe): every [128x128]xN matmul costs ~22 ns fixed +
    N/2.4GHz streaming, weight dtype irrelevant (LDWEIGHTS hidden).
    Per step: 64 main MMs (N=128) 4821 + 2 seed MMs (N=512) 470 +
    y MM 75 = ~5366 ns theoretical.
  * Differential builds (HW): pure 64-MM stream runs 4411 ns/step;
    +ACT w/o recurrence chain 4611; +chain 6284 with 4 pair-ACTs,
    5430 with 2 half-ACTs -- each wait-carrying PE matmul (cross-step
    ACT->PE edge) costs ~430 ns (sem check + 173 ns SBUF-pipeline
    refill + decode break), so fewer, coarser edges win. One whole-step
    ACT would serialize ~1.2 us into the chain; two is the sweet spot.
  * History: fp16 B=32 baseline 1.357 ms -> y add-tree 2396 ns/step ->
    2-way time split B=64 1.015 ms -> 4-way split B=128 2-step phases
    6166 ns/step x 160 = 986 us (this file's predecessor).
  * Tried and rejected (HW-measured): fp8e3 weights (no gain, not
    weight-load bound); explicit ldweights (2x worse); U=128 (no change);
    8-way split w/ 1-step phases (13.7 us/step -- seed MMs on critical
    path); cross-core H-split (D2D 0.5-2us/hop >> step budget);
    U=48 with phase-split (6284 vs 6121 ns/step -- larger unroll hurts
    fetch); 8-way split REBUILT with this file's full structure
    (ring-2, phase-split, 1-ahead seeds, zbuf epilogue; see kernel8.py):
    correct at 4.3e-4 but 11784 ns/step x 80 = 943 us -- the N=256
    config's chain/contention overhead (~3 us/step over its 8.7 us PE
    floor) eats the entire per-MM fixed-cost saving. fp8e4 DoubleRow
    (0.5 cyc/row) is numerically dead: e4m3 ~6-12% steps >> 2e-2 gate.
"""

import numpy as np

import concourse.bass as bass
import concourse.bacc as bacc
import concourse.mybir as mybir
import concourse.tile as tile
from concourse.bass_utils import run_bass_kernel_spmd

P = 128          # partitions
B = 128          # local batch (2 batch-groups x 4 time-quarters)
H = 1024
NCH = 8          # H / P chunks
T_FULL = 512
BURN = 16        # time-split burn-in steps (err ~5e-6 at 16, HW-measured)
U = 24           # steps per hardware-loop iteration (must be % 12 == 0)
N_CORES = 8

DT_MAIN = mybir.dt.float16
NP_MAIN = np.float16

_CACHE = {}


def _build(t_total: int, debug: bool, repeat: int = 1):
    """Build the Bass program for one core. Returns nc."""
    key = (t_total, debug, DT_MAIN, repeat)
    if key in _CACHE:
        return _CACHE[key]

    assert t_total % U == 0
    niter = t_total // U
    f32 = mybir.dt.float32

    nc = bacc.Bacc("TRN2", target_bir_lowering=False, debug=debug)

    # ---- DRAM I/O ----
    w_d = nc.dram_tensor("w", [P, NCH * NCH * P], DT_MAIN, kind="ExternalInput")
    # seed lhsT: rows 2i/2i+1 = [W_ih; c] of chunk 4s+i, cols s*128+m
    seedw_d = nc.dram_tensor("seedw", [8, 2 * P], DT_MAIN, kind="ExternalInput")
    # block-diag seed rhs per step: [8, 512], rows 2i/2i+1 = [x_t; 1] at
    # col block i (chunk 4s+i), zeros elsewhere; padded 8 extra steps
    x_d = nc.dram_tensor("x", [8, (t_total + 8) * 512], DT_MAIN,
                         kind="ExternalInput")
    h0_d = nc.dram_tensor("h0", [H, B], DT_MAIN, kind="ExternalInput")
    # wout pre-broadcast to the hT layout: wout[p, o*128+b] = W_out[o*128+p]
    wout_d = nc.dram_tensor("wout", [P, NCH * B], f32, kind="ExternalInput")
    # 512-col guard at the front: the y pipeline runs one step behind the
    # recurrence, so the first iteration's copy lands at offset -512.
    y_d = nc.dram_tensor("y", [1, t_total * B + 512], f32,
                         kind="ExternalOutput")

    with tile.TileContext(nc) as tc:
        with (
            tc.tile_pool(name="pool", bufs=1) as pool,
            tc.tile_pool(name="psum", bufs=1, space=bass.MemorySpace.PSUM) as psum,
        ):
            w_sb = pool.tile([P, NCH * NCH * P], DT_MAIN, tag="w")
            seedw_sb = pool.tile([8, 2 * P], DT_MAIN, tag="seedw")
            wout_sb = pool.tile([P, NCH * B], f32, tag="wout")
            hT = [
                pool.tile([P, NCH * B], DT_MAIN, tag="hT0", name="hT0"),
                pool.tile([P, NCH * B], DT_MAIN, tag="hT1", name="hT1"),
            ]
            prod = pool.tile([P, NCH * B], f32, tag="prod")
            z_sb = pool.tile([P, 8 * B], DT_MAIN, tag="z")   # 8 step slots
            y_sb = pool.tile([1, 2 * 512], f32, tag="ysb")   # 2 4-step slots
            # x ring: 2 slots of 4 steps each (one 32KB DMA per 4 steps)
            x_ring = pool.tile([8, 2 * 2048], DT_MAIN, tag="xring")
            ones_sb = pool.tile([P, 1], DT_MAIN, tag="ones")

            # 3-slot main psum ring (2 banks each) + 2-bank y ring = 8 banks
            mb = [
                psum.tile([P, NCH * B], f32, tag=f"mb{i}", name=f"mb{i}")
                for i in range(3)
            ]
            y_ps = psum.tile([P, 1024], f32, tag="yps", name="yps")

            def seed_mms(slot, xcol0):
                """Seed psum slot (one future step) with W_ih x + bias.
                One start=True matmul per bank (N=512, K=8)."""
                for s in range(2):
                    nc.tensor.matmul(
                        mb[slot][:, s * 512:(s + 1) * 512],
                        seedw_sb[:, s * P:(s + 1) * P],
                        x_ring[:, xcol0:xcol0 + 512],
                        start=True,
                        stop=False,
                        skip_group_check=True,
                    )

            # ---- prologue ----
            nc.sync.dma_start(out=w_sb[:], in_=w_d[:])
            nc.sync.dma_start(out=seedw_sb[:], in_=seedw_d[:])
            nc.sync.dma_start(out=wout_sb[:], in_=wout_d[:])
            nc.sync.dma_start(
                out=hT[0][:].rearrange("p (k b) -> p k b", k=NCH),
                in_=h0_d[:].rearrange("(k p) b -> p k b", p=P),
            )
            nc.gpsimd.memset(ones_sb[:], 1.0)
            nc.gpsimd.memset(z_sb[:], 0.0)
            for s in range(2):
                nc.sync.dma_start(
                    out=x_ring[:, s * 2048:(s + 1) * 2048],
                    in_=x_d[:, s * 2048:(s + 1) * 2048],
                )
            seed_mms(0, 0)      # step 0
            seed_mms(1, 512)    # step 1

            # ---- main loop ----
            from contextlib import nullcontext

            rep_cm = (tc.For_i(0, repeat, 1, name="rep") if repeat > 1
                      else nullcontext(0))
            with rep_cm, tc.For_i(
                    0, niter, 1, hint_engines=(mybir.EngineType.PE,)) as g:
                for j in range(U):
                    slot = j % 3
                    h_in = hT[j % 2]
                    h_out = hT[(j + 1) % 2]

                    # x prefetch: one 32KB DMA per 4 steps (group j//4 + 2),
                    # issued at j%4==2, right after the slot's last seed read
                    if j % 4 == 2:
                        xs0 = ((j // 4) % 2) * 2048
                        nc.sync.dma_start(
                            out=x_ring[:, xs0:xs0 + 2048],
                            in_=x_d[:, bass.ds(
                                g * (U * 512) + (4 * (j // 4) + 8) * 512, 2048)],
                        )

                    # h_preT[o] += sum_k Wt(k,o).T @ hT_k, split into two
                    # fixed phases: Phase-E contracts chunks 0-3 (tanh'd
                    # EARLY in step j-1, mid-Phase-L), Phase-L chunks 4-7
                    # (tanh'd at step j-1's end).  tanh fires as TWO
                    # half-ACTs (512 cols = one psum bank each) inside
                    # Phase-L as soon as a half's output pairs finish.  Each
                    # wait-carrying PE matmul costs ~430ns (HW-measured), so
                    # two coarse cross-step edges instead of four, and the
                    # phase split gives both >= 1us of slack (the old
                    # interleaved k_order consumed late-half chunks at MM#9,
                    # ~620ns in, vs producer ready ~640ns -- marginal every
                    # step).  One whole-step ACT would serialize ~1.2us, so
                    # two is the sweet spot.
                    for a in range(4):
                        for k in range(4):
                            for half in range(2):
                                o = 2 * a + half
                                nc.tensor.matmul(
                                    mb[slot][:, o * P:(o + 1) * P],
                                    w_sb[:, (k * NCH + o) * P:(k * NCH + o + 1) * P],
                                    h_in[:, k * P:(k + 1) * P],
                                    start=False,
                                    stop=False,
                                    skip_group_check=True,
                                )

                    # seeds for step j+2, emitted between the phases: their
                    # WAR wait (slot (j+2)%3 was read by ACT(j-1)) rides the
                    # same ACT semaphore count Phase-L needs, and by
                    # mid-step it is long satisfied
                    seed_mms((j + 2) % 3,
                             (((j + 2) // 4) % 2) * 2048 + ((j + 2) % 4) * 512)

                    for a in range(4):
                        for k in range(4, 8):
                            for half in range(2):
                                o = 2 * a + half
                                nc.tensor.matmul(
                                    mb[slot][:, o * P:(o + 1) * P],
                                    w_sb[:, (k * NCH + o) * P:(k * NCH + o + 1) * P],
                                    h_in[:, k * P:(k + 1) * P],
                                    start=False,
                                    stop=(k == 7),
                                    skip_group_check=True,
                                )
                        if a == 1:
                            nc.scalar.activation(
                                h_out[:, 0:512],
                                mb[slot][:, 0:512],
                                mybir.ActivationFunctionType.Tanh,
                            )
                        elif a == 3:
                            nc.scalar.activation(
                                h_out[:, 512:1024],
                                mb[slot][:, 512:1024],
                                mybir.ActivationFunctionType.Tanh,
                            )

                    # z[p, b] = sum_o wout[p, o*128+b] * h_out[p, o*128+b]
                    nc.vector.tensor_mul(prod[:], wout_sb[:], h_out[:])
                    nc.vector.tensor_add(
                        prod[:, 0:512], prod[:, 0:512], prod[:, 512:1024])
                    nc.vector.tensor_add(
                        prod[:, 0:256], prod[:, 0:256], prod[:, 256:512])
                    zc = (j % 8) * B
                    nc.vector.tensor_add(
                        z_sb[:, zc:zc + B], prod[:, 0:B], prod[:, B:2 * B])
                    if j % 4 == 0:
                        # partition-reduce z of steps j-4..j-1 on PE (a full
                        # step behind the DVE tree; only the first matmul of
                        # the burst carries the DVE wait). Requires U%8==0.
                        b0 = ((j // 4 + 1) % 2) * 4
                        for r in range(4):
                            nc.tensor.matmul(
                                y_ps[0:1, (b0 + r) * B:(b0 + r + 1) * B],
                                ones_sb[:, 0:1],
                                z_sb[:, (b0 + r) * B:(b0 + r + 1) * B],
                                start=True,
                                stop=True,
                            )
                        bh = b0 // 4
                        # copy on DVE, not ACT: keeps the ACT queue holding
                        # only the two critical-path tanh instructions
                        nc.vector.tensor_copy(
                            y_sb[0:1, bh * 512:bh * 512 + 512],
                            y_ps[0:1, bh * 512:bh * 512 + 512],
                        )
                        nc.sync.dma_start(
                            out=y_d[0:1, bass.ds(
                                g * (U * B) + (j - 4) * B + 512, 512)],
                            in_=y_sb[0:1, bh * 512:bh * 512 + 512],
                        )

            # ---- tiny epilogue: y for the final 4 steps ----
            for r in range(4):
                nc.tensor.matmul(
                    y_ps[0:1, (4 + r) * B:(4 + r + 1) * B],
                    ones_sb[:, 0:1],
                    z_sb[:, (4 + r) * B:(4 + r + 1) * B],
                    start=True,
                    stop=True,
                )
            nc.vector.tensor_copy(
                y_sb[0:1, 512:1024],
                y_ps[0:1, 512:1024],
            )
            nc.sync.dma_start(
                out=y_d[0:1, (t_total - 4) * B + 512:t_total * B + 512],
                in_=y_sb[0:1, 512:1024],
            )

    nc.compile()
    _CACHE[key] = nc
    return nc


def _prep_inputs(initial_input, hidden, targets, W_ih, W_hh, b_ih, b_hh,
                 W_out, t_full):
    """Host-side prep: returns the 8 per-core input maps.

    Core ci = (ts, bg): time-quarter ts = ci // 2, batch-group bg = ci % 2.
    ts=0 runs steps [0, t_core); ts>0 runs steps [ts*t_full/4 - BURN, ...),
    starting from h=0 -- the teacher-forced recurrence forgets its initial
    state (err ~5e-6 after 16 steps), so the first BURN outputs of ts>0
    are discarded.
    """
    f32 = np.float32
    t_core = t_full // 4 + BURN
    # x sequence: teacher-forced input at step t is targets[t-1], x_0 = initial
    x_seq = np.concatenate(
        [np.asarray(initial_input, f32).reshape(1, -1),
         np.asarray(targets, f32)[: t_full - 1, :, 0]],
        axis=0,
    )  # [T, 256]
    c_bias = (np.asarray(b_ih, f32) + np.asarray(b_hh, f32))
    # w_sb[p, (k*8+o)*128+m] = W_hh.T[k*128+p, o*128+m]
    wt = (
        np.asarray(W_hh, f32).T.reshape(NCH, P, NCH, P)
        .transpose(1, 0, 2, 3)
        .reshape(P, NCH * NCH * P)
        .astype(NP_MAIN)
    )
    # seedw[2i+t, s*128+m] = [W_ih; c][t] at H-index (4s+i)*128+m
    wih = np.asarray(W_ih, f32)[:, 0].reshape(NCH, P)
    cb = c_bias.reshape(NCH, P)
    seedw = np.zeros((8, 2 * P), f32)
    for s in range(2):
        for i in range(4):
            seedw[2 * i, s * P:(s + 1) * P] = wih[4 * s + i]
            seedw[2 * i + 1, s * P:(s + 1) * P] = cb[4 * s + i]
    seedw = seedw.astype(NP_MAIN)
    wout8 = np.asarray(W_out, f32)[0].reshape(NCH, P).T                  # [128, 8]
    wout = np.ascontiguousarray(
        np.broadcast_to(wout8[:, :, None], (P, NCH, B)).reshape(P, NCH * B)
    )

    in_maps = []
    for ci in range(N_CORES):
        ts, bg = ci // 2, ci % 2
        sl = slice(bg * B, (bg + 1) * B)
        start = 0 if ts == 0 else (ts * (t_full // 4) - BURN)
        xs = x_seq[start:start + t_core, sl].astype(NP_MAIN)  # [t_core, 128]
        xblk = np.zeros((8, (t_core + 8) * 512), NP_MAIN)
        for i in range(4):
            v = xblk[2 * i].reshape(t_core + 8, 4, P)
            v[:t_core, i, :] = xs
            xblk[2 * i + 1].reshape(t_core + 8, 4, P)[:, i, :] = 1.0
        if ts == 0:
            h0 = np.ascontiguousarray(
                np.asarray(hidden, f32)[sl].T).astype(NP_MAIN)
        else:
            h0 = np.zeros((H, B), NP_MAIN)
        in_maps.append({
            "w": wt, "seedw": seedw, "x": xblk, "h0": h0, "wout": wout,
        })
    return in_maps


def kernel(initial_input, hidden, targets, W_ih, W_hh, b_ih, b_hh, W_out,
           b_out, teacher_force_probability=None, _trace=False):
    t_full = int(np.asarray(targets).shape[0])
    t_core = t_full // 4 + BURN
    nc = _build(t_core, debug=False)
    in_maps = _prep_inputs(initial_input, hidden, targets, W_ih, W_hh, b_ih,
                           b_hh, W_out, t_full)
    res = run_bass_kernel_spmd(nc, in_maps, core_ids=list(range(N_CORES)),
                               trace=_trace)
    y = np.zeros((t_full, 256), np.float32)
    q4 = t_full // 4
    for ci, r in enumerate(res.results):
        ts, bg = ci // 2, ci % 2
        yc = r["y"].reshape(-1)[512:].reshape(t_core, B).astype(np.float32)
        if ts == 0:
            y[:q4, bg * B:(bg + 1) * B] = yc[:q4]
        else:
            y[ts * q4:(ts + 1) * q4, bg * B:(bg + 1) * B] = yc[BURN:]
    y = y + np.float32(np.asarray(b_out).reshape(-1)[0])
    out = y[:, :, None]
    if _trace:
        return out, res
    return out
